# revision 26
# baseline (speedup 1.0000x reference)
"""Trainium2 Bass kernel for nn_CustomABlock (MDTA transformer block).

Per-core layout: one batch image [C=256, N=4096(=64x64)] per NeuronCore,
data-parallel over B=8 across 8 cores, all params replicated.

Key design points:
  - qkv = W_qkv @ x on PE (bf16), drained once to fp8-e4m3 SBUF.
  - dwconv 3x3: 8 of 9 taps run on PE as 4 fp8 DoubleRow "pair" diag
    matmuls per 512-col PSUM tile (two shifted reads of the same qkv tile
    packed via a strided AP view); center tap is fused into the DVE
    PSUM-drain STT.  Row-wrap edge columns fixed by 6 small DVE STTs.
  - q is NOT normalized explicitly: 1/||q|| * temperature is folded into
    the softmax exp() per-partition scale.  k is scaled in-place (1 TS).
  - attn@v and the 1x1 proj are fused: B_g = (A_g^T @ WpT_g) is built on
    PE (2 small matmuls), then proj_out = sum_g B_g^T.T @ v_g.
  - residual path is bf16 (no f32 x input); output DMA'd as bf16 and
    upcast on host.
"""

import os
import numpy as np
import ml_dtypes

KCUT = int(os.environ.get("KCUT", "5"))  # debug: 1=qkv+taps(0) .. 5=full
# bit0: TTR norms (InstTensorTensorReduce crashes TRN2 hw — keep off),
# bit1: DVE kT copy
KFIN = int(os.environ.get("KFIN", "2"))
KSQ = int(os.environ.get("KSQ", "1"))    # 1 = squares via DVE STT accum
# 1 = wrap fixes on GPSIMD (walrus rejects Pool-engine TensorScalarPtr:
# "Instruction engine check failed" — keep on DVE)
KGW = int(os.environ.get("KGW", "0"))
WSCALE = 16.0                            # fp8 weight pre-scale (dense)
TSCALE = 8.0                             # fp8 tap-weight pre-scale

BF16 = ml_dtypes.bfloat16
F8E4 = ml_dtypes.float8_e4m3

C = 256          # dim
N = 4096         # 64*64
H = W = 64
NH = 8           # heads
HID = 307        # mlp hidden
NB = 6           # qkv channel blocks of 128
TS = 512

# tap pairs: two (dy,dx) taps per fp8 DoubleRow matmul; shift s = 64*dy+dx
PAIRS = [((-1, -1), (-1, 1)),   # s = -65, -63
         ((-1, 0), (1, 0)),     # s = -64, +64
         ((0, -1), (0, 1)),     # s = -1, +1
         ((1, -1), (1, 1))]     # s = 63, 65

_CACHE = {}


def _build_bass():
    import concourse.bass as bass
    from concourse import bacc
    from concourse import mybir
    from concourse.tile import TileContext
    from concourse.masks import make_identity

    dt = mybir.dt
    f32 = dt.float32
    bf16 = dt.bfloat16
    f8e4 = dt.float8e4
    AF = mybir.ActivationFunctionType
    OP = mybir.AluOpType
    DR = mybir.MatmulPerfMode.DoubleRow

    nc = bacc.Bacc("TRN2")

    # ---- DRAM I/O (per-core) ----
    xb_d = nc.dram_tensor("xb", [128, 2, N], bf16, kind="ExternalInput")
    x8_d = nc.dram_tensor("x8", [128, 2, N], f8e4, kind="ExternalInput")
    wqkv_d = nc.dram_tensor("wqkvT", [128, NB, 2, 128], f8e4,
                        kind="ExternalInput")
    wm18_d = nc.dram_tensor("wm18T", [128, 3, 2, 128], f8e4,
                        kind="ExternalInput")
    wpair_d = nc.dram_tensor("wpair", [128, NB, 4, 2, 128], f8e4,
                             kind="ExternalInput")
    wcen_d = nc.dram_tensor("wcen", [128, NB], f32, kind="ExternalInput")
    wwrap_d = nc.dram_tensor("wwrap", [128, NB, 6], f32, kind="ExternalInput")
    wproj_d = nc.dram_tensor("wprojT", [128, 2, C], bf16, kind="ExternalInput")
    wm2_d = nc.dram_tensor("wm2T", [128, 3, C], bf16, kind="ExternalInput")
    b1_d = nc.dram_tensor("b1", [128, 3], f32, kind="ExternalInput")
    b2_d = nc.dram_tensor("b2", [128, 2], f32, kind="ExternalInput")
    tv_d = nc.dram_tensor("tempvec", [128, 2], f32, kind="ExternalInput")
    out_d = nc.dram_tensor("out", [128, 2, N], bf16, kind="ExternalOutput")

    def pair_view(base_ap, delta, ln):
        # free dims [2, ln] where half h reads at +h*delta: DoubleRow k-tiles
        return bass.AP(base_ap.tensor, base_ap.offset,
                       [list(base_ap.ap[0]), [delta, 2], [1, ln]])

    with TileContext(nc) as tc:
        with (
            tc.tile_pool(name="wpool", bufs=1) as wpool,
            tc.tile_pool(name="xpool", bufs=1) as xpool,
            tc.tile_pool(name="qkvp", bufs=2) as qkvp,
            tc.tile_pool(name="dwqk", bufs=3) as dwqk_p,
            tc.tile_pool(name="dwv", bufs=2) as dwv_p,
            tc.tile_pool(name="scr", bufs=2) as scr_p,
            tc.tile_pool(name="qt", bufs=1) as qt_p,
            tc.tile_pool(name="small", bufs=24) as small_p,
            tc.tile_pool(name="apool", bufs=2) as a_p,
            tc.tile_pool(name="btp", bufs=2) as bt_p,
            tc.tile_pool(name="ysp", bufs=6) as ys_p,
            tc.tile_pool(name="xop", bufs=4) as xo_p,
            tc.tile_pool(name="pq", bufs=2, space="PSUM") as pq_p,
            tc.tile_pool(name="pt", bufs=2, space="PSUM") as pt_p,
            tc.tile_pool(name="psml", bufs=2, space="PSUM") as psml_p,
        ):
            # ---- load weights & x (critical path first) ----
            xb_s = xpool.tile([128, 2, N], bf16)
            x8_s = xpool.tile([128, 2, N], f8e4)
            x18_s = xpool.tile([128, 2, N], f8e4)
            wqkv_s = wpool.tile([128, NB, 2, 128], f8e4)
            nc.sync.dma_start(out=wqkv_s, in_=wqkv_d[:, :, :, :])
            for kb in range(2):
                for hf in range(2):
                    nc.sync.dma_start(
                        out=x8_s[:, kb, hf * 2048:(hf + 1) * 2048],
                        in_=x8_d[:, kb, hf * 2048:(hf + 1) * 2048])
            wpair_s = wpool.tile([128, NB, 4, 2, 128], f8e4)
            nc.sync.dma_start(out=wpair_s, in_=wpair_d[:, :, :, :, :])
            wcen_s = wpool.tile([128, NB], f32)
            nc.sync.dma_start(out=wcen_s, in_=wcen_d[:, :])
            wwrap_s = wpool.tile([128, NB, 6], f32)
            nc.sync.dma_start(out=wwrap_s, in_=wwrap_d[:, :, :])
            wproj_s = wpool.tile([128, 2, C], bf16)
            nc.sync.dma_start(out=wproj_s, in_=wproj_d[:, :, :])
            wm1_s = wpool.tile([128, 3, 2, 128], f8e4)
            nc.sync.dma_start(out=wm1_s, in_=wm18_d[:, :, :, :])
            wm2_s = wpool.tile([128, 3, C], bf16)
            nc.sync.dma_start(out=wm2_s, in_=wm2_d[:, :, :])
            for kb in range(2):
                nc.sync.dma_start(out=xb_s[:, kb, :], in_=xb_d[:, kb, :])
            b1_s = wpool.tile([128, 3], f32)
            nc.sync.dma_start(out=b1_s, in_=b1_d[:, :])
            b2_s = wpool.tile([128, 2], f32)
            nc.sync.dma_start(out=b2_s, in_=b2_d[:, :])
            tv_s = wpool.tile([128, 2], f32)
            nc.sync.dma_start(out=tv_s, in_=tv_d[:, :])

            ident = wpool.tile([128, 128], bf16)
            make_identity(nc, ident)

            # warm the PE HAM clock gate while input DMAs are in flight
            for _ in range(24):
                wu = psml_p.tile([128, TS], bf16, tag="tp")
                for i in range(4):
                    nc.tensor.transpose(wu[:, i * 128:(i + 1) * 128], ident,
                                        ident)

            qT_s = qt_p.tile([128, 32, C], bf16, tag="qT")
            kT_s = qt_p.tile([128, 32, C], bf16, tag="kT")
            dw_tiles = [None] * NB
            rq_v = [None, None]      # exp scale per group (tau/||q||)
            rqn_v = [None, None]     # negated
            Bt_v = [None, None]      # fused proj lhsT per group

            def do_qkv_taps(b):
                # qkv = W_qkv @ x -> PSUM -> fp8 SBUF (single drain)
                qkv8 = qkvp.tile([128, N], f8e4, tag="qkv", name=f"qkv{b}")
                for t in range(4):
                    pq_t = pq_p.tile([128, 1024], f32, tag="pq")
                    for hh in range(2):
                        nc.tensor.matmul(
                            pq_t[:, hh * TS:(hh + 1) * TS],
                            lhsT=wqkv_s[:, b],
                            rhs=x8_s[:, :, t * 1024 + hh * TS:
                                     t * 1024 + (hh + 1) * TS],
                            start=True, stop=True, perf_mode=DR,
                        )
                    nc.scalar.activation(
                        out=qkv8[:, t * 1024:(t + 1) * 1024], in_=pq_t,
                        func=AF.Copy, scale=1.0 / WSCALE)

                # dwconv: 4 fp8 DoubleRow pair-matmuls per 512-tile + strips
                dw_t = (dwqk_p if b < 4 else dwv_p).tile(
                    [128, N], bf16, tag=("dwqk" if b < 4 else "dwv"),
                    name=f"dw{b}")
                dw_tiles[b] = dw_t
                for t8 in range(8):
                    o0, o1 = t8 * TS, (t8 + 1) * TS
                    pt_t = pt_p.tile([128, TS], f32, tag="pt")
                    mms = []
                    for pr, ((dy1, dx1), (dy2, dx2)) in enumerate(PAIRS):
                        s1, s2 = 64 * dy1 + dx1, 64 * dy2 + dx2
                        a = max(o0, -s1)
                        bb = min(o1, N - s2)
                        if a < bb:
                            rhs = pair_view(qkv8[:, a + s1:a + s1 + (bb - a)],
                                            s2 - s1, bb - a)
                            mms.append((wpair_s[:, b, pr], rhs,
                                        (a - o0, bb - o0), DR))
                        for hh, s in ((0, s1), (1, s2)):
                            lo = max(o0, -s)
                            hi = min(o1, N - s)
                            segs = []
                            if a < bb:
                                if lo < a:
                                    segs.append((lo, min(hi, a)))
                                if hi > bb:
                                    segs.append((max(lo, bb), hi))
                            elif lo < hi:
                                segs.append((lo, hi))
                            for (u, v) in segs:
                                mms.append((wpair_s[:, b, pr, hh],
                                            qkv8[:, u + s:v + s],
                                            (u - o0, v - o0), None))
                    # a full-coverage matmul must start the PSUM group so no
                    # later partial-range op lands on pending-zero bytes
                    mms.sort(key=lambda m: m[2][0] - m[2][1])
                    assert mms[0][2] == (0, TS)
                    for i, (lhsT, rhs, (u, v), pm) in enumerate(mms):
                        nc.tensor.matmul(
                            pt_t[:, u:v], lhsT=lhsT, rhs=rhs,
                            start=(i == 0), stop=(i == len(mms) - 1),
                            perf_mode=pm,
                        )
                    # center tap + PSUM drain in one DVE op
                    nc.vector.scalar_tensor_tensor(
                        out=dw_t[:, o0:o1], in0=qkv8[:, o0:o1],
                        scalar=wcen_s[:, b:b + 1], in1=pt_t,
                        op0=OP.mult, op1=OP.add,
                    )

                # row-wrap fixes: subtract wrongly-added wrapped columns
                dw3 = dw_t.rearrange("p (y x) -> p y x", y=H)
                qk3 = qkv8.rearrange("p (y x) -> p y x", y=H)
                wrap = [
                    # (j, out y-range, in y-range, in col, out col)
                    (0, 2, 64, 0, 62, 63, 0),    # dy=-1 tap(-1,-1) @ x=0
                    (1, 1, 64, 0, 63, 63, 0),    # dy=0  tap(0,-1)  @ x=0
                    (2, 0, 64, 0, 64, 63, 0),    # dy=+1 tap(1,-1)  @ x=0
                    (3, 0, 64, 0, 64, 0, 63),    # dy=-1 tap(-1,+1) @ x=63
                    (4, 0, 63, 1, 64, 0, 63),    # dy=0  tap(0,+1)  @ x=63
                    (5, 0, 62, 2, 64, 0, 63),    # dy=+1 tap(1,+1)  @ x=63
                ]
                eng = nc.gpsimd if KGW else nc.vector
                for (j, oy0, oy1, iy0, iy1, ic, oc) in wrap:
                    eng.scalar_tensor_tensor(
                        out=dw3[:, oy0:oy1, oc:oc + 1],
                        in0=qk3[:, iy0:iy1, ic:ic + 1],
                        scalar=wwrap_s[:, b, j:j + 1],
                        in1=dw3[:, oy0:oy1, oc:oc + 1],
                        op0=OP.mult, op1=OP.add,
                    )

            def do_finish(b):
                # q/k blocks: channel norms; k scaled in place; transpose
                dw_t = dw_tiles[b]
                sq = scr_p.tile([128, N], bf16, tag="sqscr")
                ssq = small_p.tile([128, 1], f32, tag=f"ssq{b}")
                if KFIN & 1:
                    nc.vector.tensor_tensor_reduce(
                        out=sq, in0=dw_t, in1=dw_t, scale=1.0, scalar=0.0,
                        op0=OP.mult, op1=OP.add, accum_out=ssq)
                elif KSQ:
                    nc.vector.scalar_tensor_tensor(
                        out=sq, in0=dw_t, scalar=1.0, in1=dw_t,
                        op0=OP.mult, op1=OP.mult, accum_out=ssq)
                else:
                    nc.scalar.activation(out=sq, in_=dw_t, func=AF.Square,
                                         accum_out=ssq)
                nrm = small_p.tile([128, 1], f32, tag=f"nrm{b}")
                nc.scalar.sqrt(nrm, ssq)
                rn = small_p.tile([128, 1], f32, tag=f"rn{b}")
                nc.vector.reciprocal(rn, nrm)
                if b < 2:    # q: fold tau/||q|| into the softmax exp scale
                    rq = small_p.tile([128, 1], f32, tag=f"rq{b}")
                    nc.vector.tensor_mul(rq, rn, tv_s[:, b:b + 1])
                    rqn = small_p.tile([128, 1], f32, tag=f"rqn{b}")
                    nc.vector.tensor_scalar_mul(rqn, rq, -1.0)
                    rq_v[b], rqn_v[b] = rq, rqn
                else:        # k: scale rows in place
                    nc.vector.tensor_scalar_mul(dw_t, dw_t, rn)
                dst = qT_s if b < 2 else kT_s
                cof = (b % 2) * 128
                for g8 in range(8):
                    tp_t = psml_p.tile([128, TS], bf16, tag="tp")
                    for i in range(4):
                        nb = g8 * 4 + i
                        nc.tensor.transpose(
                            tp_t[:, i * 128:(i + 1) * 128],
                            dw_t[:, nb * 128:(nb + 1) * 128], ident)
                    cp = (nc.scalar.copy if (b < 2 or not (KFIN & 2))
                          else nc.vector.tensor_copy)
                    cp(out=dst[:, g8 * 4:g8 * 4 + 4, cof:cof + 128],
                       in_=tp_t.rearrange("p (a b) -> p a b", a=4))

            def do_gram(g):
                co = g * 128
                pg = psml_p.tile([128, 128], f32, tag="tp")
                for nb in range(32):
                    nc.tensor.matmul(
                        pg, lhsT=qT_s[:, nb, co:co + 128],
                        rhs=kT_s[:, nb, co:co + 128],
                        start=(nb == 0), stop=(nb == 31),
                    )
                A_t = a_p.tile([128, 128], bf16, tag="A", name=f"A{g}")
                nc.vector.memset(A_t, 0.0)
                mx = small_p.tile([128, 1], f32, tag=f"mx{g}")
                sm = small_p.tile([128, 1], f32, tag=f"sm{g}")
                ebias = small_p.tile([128, 1], f32, tag=f"eb{g}")
                for hq in range(4):
                    r0, r1 = hq * 32, hq * 32 + 32
                    nc.vector.tensor_reduce(
                        out=mx[r0:r1, :], in_=pg[r0:r1, r0:r1],
                        axis=mybir.AxisListType.X, op=OP.max)
                nc.vector.tensor_mul(ebias, mx, rqn_v[g])
                for hq in range(4):
                    r0, r1 = hq * 32, hq * 32 + 32
                    nc.scalar.activation(
                        out=A_t[r0:r1, r0:r1], in_=pg[r0:r1, r0:r1],
                        func=AF.Exp, bias=ebias[r0:r1, :],
                        scale=rq_v[g][r0:r1, :], accum_out=sm[r0:r1, :])
                rs = small_p.tile([128, 1], f32, tag=f"rs{g}")
                nc.vector.reciprocal(rs, sm)
                nc.vector.tensor_scalar_mul(A_t, A_t, rs)
                # fused proj: Bt_g = A_g^T @ WpT_g  -> [d, o] lhsT for tail
                pb = psml_p.tile([128, C], f32, tag="tp")
                nc.tensor.matmul(pb, lhsT=A_t, rhs=wproj_s[:, g, :],
                                 start=True, stop=True)
                Bt = bt_p.tile([128, C], bf16, tag="Bt", name=f"Bt{g}")
                nc.scalar.copy(out=Bt, in_=pb)
                Bt_v[g] = Bt

            if KCUT >= 5:
                do_qkv_taps(0)
                do_qkv_taps(2)
                do_finish(0)
                do_qkv_taps(1)
                do_finish(2)
                do_gram(0)
                do_qkv_taps(3)
                do_finish(1)
                do_qkv_taps(4)
                do_finish(3)
                do_gram(1)
                do_qkv_taps(5)
            else:
                do_qkv_taps(0)
                if KCUT >= 2:
                    for b in (2, 1, 3, 4, 5):
                        do_qkv_taps(b)
                if KCUT >= 3:
                    for b in (0, 2, 1, 3):
                        do_finish(b)
                if KCUT >= 4:
                    do_gram(0)
                    do_gram(1)
                xo_dbg = xo_p.tile([128, 1024], bf16, tag="xo")
                nc.vector.tensor_copy(out=xo_dbg, in_=dw_tiles[0][:, 0:1024])
                nc.sync.dma_start(out=out_d[:, 0, 0:1024], in_=xo_dbg)

            # ---- streamed tail: (attn@v+proj)+resid / mlp1+gelu /
            #      mlp2+resid2 -> bf16 out DMA, per 1024-col tile ----
            for t in range(4 if KCUT >= 5 else 0):
                sl = slice(t * 1024, (t + 1) * 1024)
                for ob in range(2):
                    pp = pq_p.tile([128, 1024], f32, tag="pq")
                    for hh in range(2):
                        for g in range(2):
                            nc.tensor.matmul(
                                pp[:, hh * TS:(hh + 1) * TS],
                                lhsT=Bt_v[g][:, ob * 128:(ob + 1) * 128],
                                rhs=dw_tiles[4 + g][:, t * 1024 + hh * TS:
                                                    t * 1024 + (hh + 1) * TS],
                                start=(g == 0), stop=(g == 1))
                    nc.vector.scalar_tensor_tensor(
                        out=xb_s[:, ob, sl], in0=pp, scalar=1.0 / TSCALE,
                        in1=xb_s[:, ob, sl], op0=OP.mult, op1=OP.add)
                    # fp8 copy of x1 for the mlp1 DoubleRow matmul
                    nc.vector.tensor_copy(out=x18_s[:, ob, sl],
                                          in_=xb_s[:, ob, sl])
                ys = []
                for mb in range(3):
                    rows = 128 if mb < 2 else HID - 256
                    pm = pq_p.tile([128, 1024], f32, tag="pq")
                    for hh in range(2):
                        nc.tensor.matmul(
                            pm[:rows, hh * TS:(hh + 1) * TS],
                            lhsT=wm1_s[:, mb, :, 0:rows],
                            rhs=x18_s[:, :, t * 1024 + hh * TS:
                                      t * 1024 + (hh + 1) * TS],
                            start=True, stop=True, perf_mode=DR)
                    ys_t = ys_p.tile([128, 1024], bf16, tag="ys")
                    nc.scalar.activation(
                        out=ys_t[:rows, :], in_=pm[:rows, :],
                        func=AF.Gelu_apprx_tanh, bias=b1_s[:rows, mb:mb + 1],
                        scale=1.0 / WSCALE)
                    ys.append(ys_t)
                for ob in range(2):
                    xo_t = xo_p.tile([128, 1024], bf16, tag="xo")
                    for hh in range(2):
                        pm2 = pt_p.tile([128, TS], f32, tag="pt")
                        for kb in range(3):
                            rows = 128 if kb < 2 else HID - 256
                            nc.tensor.matmul(
                                pm2[:, :],
                                lhsT=wm2_s[:rows, kb, ob * 128:(ob + 1) * 128],
                                rhs=ys[kb][:rows, hh * TS:(hh + 1) * TS],
                                start=(kb == 0), stop=(kb == 2))
                        nc.vector.scalar_tensor_tensor(
                            out=xo_t[:, hh * TS:(hh + 1) * TS], in0=pm2,
                            scalar=b2_s[:, ob:ob + 1],
                            in1=xb_s[:, ob, t * 1024 + hh * TS:
                                     t * 1024 + (hh + 1) * TS],
                            op0=OP.add, op1=OP.add)
                    nc.sync.dma_start(out=out_d[:, ob, sl], in_=xo_t)

    return nc


def _prep_shared(w_qkv, w_dw, temperature, w_proj, w_mlp1, b_mlp1, w_mlp2,
                 b_mlp2):
    f32 = np.float32
    shared = {}
    shared["wqkvT"] = np.ascontiguousarray(
        (WSCALE * w_qkv).T.reshape(2, 128, NB, 128).transpose(1, 2, 0, 3)
    ).astype(F8E4)
    ar = np.arange(128)
    wp = np.zeros((128, NB, 4, 2, 128), f32)
    for b in range(NB):
        for pr, taps in enumerate(PAIRS):
            for h, (dy, dx) in enumerate(taps):
                wp[ar, b, pr, h, ar] = \
                    TSCALE * w_dw[b * 128 + ar, 0, dy + 1, dx + 1]
    shared["wpair"] = wp.astype(F8E4)
    wc = np.zeros((128, NB), f32)
    ww = np.zeros((128, NB, 6), f32)
    for b in range(NB):
        wc[:, b] = TSCALE * w_dw[b * 128:(b + 1) * 128, 0, 1, 1]
        for j, dy in enumerate((-1, 0, 1)):
            ww[:, b, j] = -TSCALE * w_dw[b * 128:(b + 1) * 128, 0, dy + 1, 0]
            ww[:, b, j + 3] = \
                -TSCALE * w_dw[b * 128:(b + 1) * 128, 0, dy + 1, 2]
    shared["wcen"] = wc
    shared["wwrap"] = ww
    shared["wprojT"] = np.ascontiguousarray(
        w_proj.T.reshape(2, 128, C).transpose(1, 0, 2)).astype(BF16)
    w1p = np.zeros((C, 384), f32)
    w1p[:, :HID] = (WSCALE * w_mlp1).T
    shared["wm18T"] = np.ascontiguousarray(
        w1p.reshape(2, 128, 3, 128).transpose(1, 2, 0, 3)).astype(F8E4)
    w2 = np.zeros((384, C), f32)
    w2[:HID] = w_mlp2.T
    shared["wm2T"] = np.ascontiguousarray(
        w2.reshape(3, 128, C).transpose(1, 0, 2)).astype(BF16)
    b1 = np.zeros((384,), f32)
    b1[:HID] = b_mlp1
    shared["b1"] = np.ascontiguousarray(b1.reshape(3, 128).T)
    shared["b2"] = np.ascontiguousarray(b_mlp2.astype(f32).reshape(2, 128).T)
    t = temperature.reshape(NH).astype(f32)
    tv = np.zeros((128, 2), f32)
    for g in range(2):
        tv[:, g] = np.repeat(t[g * 4:(g + 1) * 4], 32)
    shared["tempvec"] = tv
    return shared


def kernel(x, w_qkv, w_dw, temperature, w_proj, w_mlp1, b_mlp1, w_mlp2, b_mlp2,
           _trace=False):
    from concourse.bass_utils import run_bass_kernel_spmd

    if "nc" not in _CACHE:
        nc = _build_bass()
        nc.finalize()
        _CACHE["nc"] = nc
    nc = _CACHE["nc"]

    x = np.asarray(x, np.float32)
    B = x.shape[0]
    shared = _prep_shared(
        np.asarray(w_qkv, np.float32), np.asarray(w_dw, np.float32),
        np.asarray(temperature, np.float32), np.asarray(w_proj, np.float32),
        np.asarray(w_mlp1, np.float32), np.asarray(b_mlp1, np.float32),
        np.asarray(w_mlp2, np.float32), np.asarray(b_mlp2, np.float32))

    in_maps = []
    for i in range(B):
        m = dict(shared)
        xi = np.ascontiguousarray(x[i].reshape(2, 128, N).transpose(1, 0, 2))
        m["xb"] = xi.astype(BF16)
        m["x8"] = xi.astype(F8E4)
        in_maps.append(m)

    res = run_bass_kernel_spmd(nc, in_maps, core_ids=list(range(B)),
                               trace=_trace)
    outs = np.stack([
        r["out"].astype(np.float32).transpose(1, 0, 2).reshape(C, H, W)
        for r in res.results
    ])
    if _trace:
        _CACHE["last_exec_ns"] = res.exec_time_ns
        _CACHE["last_profile"] = res.profile_json
        it = res.instructions_and_trace
        _CACHE["last_trace"] = it[1] if it else None
    return outs
